# revision 1
# baseline (speedup 1.0000x reference)
import sys
if '/opt/trn_rl_repo' not in sys.path:
    sys.path.insert(0, '/opt/trn_rl_repo')
import numpy as np
import ml_dtypes

import concourse.bass as bass
import concourse.bacc as bacc
import concourse.mybir as mybir
import concourse.tile as tile
from concourse.masks import make_identity

P = 128
N_CORES = 8
LAYERS = 2
RG = [list(range(N_CORES))]

F32 = mybir.dt.float32
BF16 = mybir.dt.bfloat16
FP8 = mybir.dt.float8e4
I32 = mybir.dt.int32
NP_BF16 = ml_dtypes.bfloat16
NP_FP8 = mybir.dt.np(mybir.dt.float8e4)


def _cdiv(a, b):
    return -(-a // b)


# ---------------------------------------------------------------------------
# host-side edge packing (same scheme as baseline)
# ---------------------------------------------------------------------------

BW = 32      # scatter block width (rows per one-hot block)


def pack_edges64(rows, cols, vals, nblk64, lr_np_dt, val_np_dt):
    """Blocks of BW rows, exact per-block-position chunk counts (shared
    across cores via max, min 1). Returns per-core packed tiles plus the
    chunk schedule.
      idx/lr/val: [N_CORES, P, CH_TOT] — slot (core, lane, offs[b]+j) holds
        edge #(j*128+lane) of local block b.
      chunks[b]: 128-edge chunks for block position b (same on all cores).
    """
    nblk_total = N_CORES * nblk64
    order = np.argsort(rows, kind='stable')
    r = rows[order].astype(np.int64)
    c = cols[order].astype(np.int64)
    v = vals[order]
    blk = r // BW
    counts = np.bincount(blk, minlength=nblk_total)
    boffs = np.zeros(nblk_total + 1, np.int64)
    np.cumsum(counts, out=boffs[1:])
    rank = np.arange(len(r), dtype=np.int64) - boffs[blk]
    j = rank >> 7
    lane = rank & 127
    core = blk // nblk64
    lblk = blk % nblk64
    cnt = counts.reshape(N_CORES, nblk64)
    chunks = np.maximum(1, _cdiv_arr(cnt.max(axis=0), P))
    offs = np.zeros(nblk64 + 1, np.int64)
    np.cumsum(chunks, out=offs[1:])
    ch_tot = int(offs[-1])
    colpos = offs[lblk] + j
    idx = np.zeros((N_CORES, P, ch_tot), np.int32)
    lr = np.zeros((N_CORES, P, ch_tot), np.float32)
    val = np.zeros((N_CORES, P, ch_tot), np.float32)
    idx[core, lane, colpos] = c
    lr[core, lane, colpos] = (r % BW)
    val[core, lane, colpos] = v
    return idx, lr.astype(lr_np_dt), val.astype(val_np_dt), chunks, offs


def _cdiv_arr(a, b):
    return -(-a // b)


def make_groups(chunks, cw):
    """Greedy-group consecutive PAIRS of 64-blocks (=128-row blocks) so each
    group's total chunk count <= cw. Returns list of (B0, nB, c0, ncw):
    128-block range [B0, B0+nB), chunk-col range [c0, c0+ncw)."""
    hb = P // BW
    nb128 = len(chunks) // hb
    pair = chunks.reshape(nb128, hb).sum(axis=1)
    assert pair.max() <= cw, (pair.max(), cw)
    groups = []
    B0, acc = 0, 0
    for B in range(nb128):
        if acc and acc + pair[B] > cw:
            groups.append((B0, B - B0))
            B0, acc = B, 0
        acc += pair[B]
    groups.append((B0, nb128 - B0))
    offs = np.zeros(len(chunks) + 1, np.int64)
    np.cumsum(chunks, out=offs[1:])
    out = []
    for (B0, nB) in groups:
        c0 = int(offs[hb * B0])
        ncw = int(offs[hb * (B0 + nB)] - c0)
        out.append((B0, nB, c0, ncw))
    return out


# ---------------------------------------------------------------------------
# host-side A2A routing for the head
# ---------------------------------------------------------------------------

def build_routing(req_rows, rows_per_core):
    """req_rows: [N_CORES, n_req] global row ids requested by each dest core.
    Returns (pkc, pack_idx, pos) where
      pack_idx [N_CORES owners, N_CORES dests, pkc] int32: local row ids owner
        o must pack for dest j (pad 0),
      pos [N_CORES, n_req] int32: recv-buffer row (o*pkc + s) for each request.
    """
    nd, nr = req_rows.shape
    uniq = []          # uniq[j][o] = sorted unique local ids
    for j in range(nd):
        owner = req_rows[j] // rows_per_core
        local = req_rows[j] % rows_per_core
        uniq.append([np.unique(local[owner == o]) for o in range(N_CORES)])
    pkc = max(len(u) for js in uniq for u in js)
    pkc = _cdiv(max(pkc, 1), P) * P
    pack_idx = np.zeros((N_CORES, nd, pkc), np.int32)
    pos = np.zeros((nd, nr), np.int32)
    for j in range(nd):
        owner = req_rows[j] // rows_per_core
        local = req_rows[j] % rows_per_core
        for o in range(N_CORES):
            u = uniq[j][o]
            pack_idx[o, j, :len(u)] = u
            sel = owner == o
            if sel.any():
                s = np.searchsorted(u, local[sel])
                pos[j, sel] = o * pkc + s
    return pkc, pack_idx, pos


# ---------------------------------------------------------------------------
# bass program builder
# ---------------------------------------------------------------------------

def build_program(cfg):
    NU = cfg['NU']; NS = cfg['NS']; NM = cfg['NM']; D = cfg['D']
    NBU = cfg['NBU']; NBM = cfg['NBM']
    BT = cfg['BT']; L = cfg['L']
    CW = cfg['CW']
    PKU = cfg['PKU']; PKM = cfg['PKM']
    ui_chunks = np.asarray(cfg['ui_chunks'])
    mm_chunks = np.asarray(cfg['mm_chunks'])
    CHU_TOT = int(ui_chunks.sum())
    CHM_TOT = int(mm_chunks.sum())
    ui_groups = make_groups(ui_chunks, CW)
    mm_groups = make_groups(mm_chunks, CW)
    ui_offs = np.zeros(len(ui_chunks) + 1, np.int64)
    np.cumsum(ui_chunks, out=ui_offs[1:])
    mm_offs = np.zeros(len(mm_chunks) + 1, np.int64)
    np.cumsum(mm_chunks, out=mm_offs[1:])
    EDGE_DT = BF16      # head/A2A path dtype
    GATH_DT = FP8       # graph tables, gathered rows, one-hots, edge vals

    RWU = NBU * P
    RWM = NBM * P
    UI_ROWS = N_CORES * RWU
    MM_ROWS = N_CORES * RWM
    WU = N_CORES * PKU // P      # pack-idx cols for uif rows
    WM = N_CORES * PKM // P

    nc = bacc.Bacc("TRN2", target_bir_lowering=False, debug=False,
                   enable_asserts=False, num_devices=N_CORES)

    def din(name, shape, dt):
        return nc.dram_tensor(name, shape, dt, kind="ExternalInput").ap()

    x0f = din("x0f", [UI_ROWS, D], GATH_DT)    # full ui table (replicated)
    m0f = din("m0f", [MM_ROWS, D], GATH_DT)    # full mashup table (replicated)
    x0s = din("x0s", [RWU, D], F32)            # this core's ui row shard
    m0s = din("m0s", [RWM, D], F32)
    ui_idx = din("ui_idx", [P, CHU_TOT], I32)
    ui_lr = din("ui_lr", [P, CHU_TOT], EDGE_DT)
    ui_val = din("ui_val", [P, CHU_TOT], GATH_DT)
    mm_idx = din("mm_idx", [P, CHM_TOT], I32)
    mm_lr = din("mm_lr", [P, CHM_TOT], EDGE_DT)
    mm_val = din("mm_val", [P, CHM_TOT], GATH_DT)
    dinv = din("dinv", [P, NBM], F32)
    pack_u = din("pack_u", [P, WU], I32)       # local uif rows to send, (j,i)
    pack_m = din("pack_m", [P, WM], I32)
    memb_pos = din("memb_pos", [BT * P, L], I32)   # recv-row of member l
    svc_pos = din("svc_pos", [BT * P, 1], I32)
    mash_pos = din("mash_pos", [BT * P, 1], I32)
    mask_lb = din("mask_lb", [BT, L * P], F32)
    aw1 = din("aw1", [2 * D, 16], F32)
    ab1 = din("ab1", [16], F32)
    aw2 = din("aw2", [16, 1], F32)
    ab2 = din("ab2", [1], F32)
    pw1 = din("pw1", [3 * D, 8], F32)
    pb1 = din("pb1", [8], F32)
    pw2 = din("pw2", [8, 1], F32)
    pb2 = din("pb2", [1], F32)

    y = nc.dram_tensor("y", [BT * P, 1], F32, kind="ExternalOutput").ap()

    with tile.TileContext(nc) as tc:
        with tc.tile_pool(name="dram", bufs=1, space="DRAM") as dram, \
             tc.tile_pool(name="res", bufs=1) as res, \
             tc.tile_pool(name="accp", bufs=1) as accp:
            h1bf_in = dram.tile([RWU, D], GATH_DT)
            h1bf_full = dram.tile([UI_ROWS, D], GATH_DT, addr_space="Shared")
            m1bf_in = dram.tile([RWM, D], GATH_DT)
            m1bf_full = dram.tile([MM_ROWS, D], GATH_DT, addr_space="Shared")
            uif_in = dram.tile([RWU, D], EDGE_DT)      # this core's uif shard
            macc_in = dram.tile([RWM, D], EDGE_DT)
            su_send = dram.tile([N_CORES * PKU, D], EDGE_DT)
            ru_recv = dram.tile([N_CORES * PKU, D], EDGE_DT)
            sm_send = dram.tile([N_CORES * PKM, D], EDGE_DT)
            rm_recv = dram.tile([N_CORES * PKM, D], EDGE_DT)

            # resident SBUF
            acc_u = accp.tile([P, NBU * D], F32)
            acc_m = accp.tile([P, NBM * D], F32)
            iota_t = res.tile([P, BW], EDGE_DT)
            nc.gpsimd.iota(iota_t[:], [[1, BW]], base=0, channel_multiplier=0,
                           allow_small_or_imprecise_dtypes=True)
            iota_rep = res.tile([P, CW * BW], EDGE_DT)
            nc.gpsimd.iota(iota_rep[:], [[0, CW], [1, BW]], base=0,
                           channel_multiplier=0,
                           allow_small_or_imprecise_dtypes=True)
            uiidx_t = res.tile([P, CHU_TOT], I32)
            nc.sync.dma_start(out=uiidx_t[:], in_=ui_idx[:])
            uilr_t = res.tile([P, CHU_TOT], EDGE_DT)
            nc.sync.dma_start(out=uilr_t[:], in_=ui_lr[:])
            uival_t = res.tile([P, CHU_TOT], GATH_DT)
            nc.sync.dma_start(out=uival_t[:], in_=ui_val[:])
            mmidx_t = res.tile([P, CHM_TOT], I32)
            nc.sync.dma_start(out=mmidx_t[:], in_=mm_idx[:])
            mmlr_t = res.tile([P, CHM_TOT], EDGE_DT)
            nc.sync.dma_start(out=mmlr_t[:], in_=mm_lr[:])
            mmval_t = res.tile([P, CHM_TOT], GATH_DT)
            nc.sync.dma_start(out=mmval_t[:], in_=mm_val[:])
            dinv_t = res.tile([P, NBM], F32)
            nc.sync.dma_start(out=dinv_t[:], in_=dinv[:])

            nc.sync.dma_start(
                out=acc_u[:].rearrange("p (b d) -> p b d", d=D),
                in_=x0s.rearrange("(b p) d -> p b d", p=P))
            nc.sync.dma_start(
                out=acc_m[:].rearrange("p (b d) -> p b d", d=D),
                in_=m0s.rearrange("(b p) d -> p b d", p=P))

            # ----------------- spmm layer helper ----------------------------
            # 64-row scatter blocks: each 128-edge chunk scatters into a
            # 64-row half-block via a 64-wide one-hot; even/odd halves land
            # in pm[0:64]/pm[64:128] via PE column tiling.
            def spmm_layer(src_tbl, groups, offs, idx_t, lr_t, val_t, post):
                with tc.tile_pool(name="sp_sb", bufs=4) as sp, \
                     tc.tile_pool(name="sp_ps", bufs=8, space="PSUM") as pp:
                    for (B0, nB, c0, ncw) in groups:
                        gt = sp.tile([P, CW * D], GATH_DT, tag="gt")
                        nc.gpsimd.indirect_dma_start(
                            out=gt[:, :ncw * D],
                            out_offset=None,
                            in_=src_tbl[:],
                            in_offset=bass.IndirectOffsetOnAxis(
                                ap=idx_t[:, c0:c0 + ncw], axis=0))
                        st = sp.tile([P, CW * BW], GATH_DT, tag="st")
                        s3 = st[:, :ncw * BW].rearrange("p (k w) -> p k w", w=BW)
                        nc.vector.tensor_tensor(
                            out=s3,
                            in0=iota_rep[:, :ncw * BW]
                                .rearrange("p (k w) -> p k w", w=BW),
                            in1=lr_t[:, c0:c0 + ncw].to_broadcast([P, ncw, BW]),
                            op=mybir.AluOpType.is_equal)
                        nc.gpsimd.tensor_tensor(
                            out=s3, in0=s3,
                            in1=val_t[:, c0:c0 + ncw].to_broadcast([P, ncw, BW]),
                            op=mybir.AluOpType.mult)
                        HB = P // BW
                        for B in range(B0, B0 + nB):
                            pm = pp.tile([P, D], F32, tag="pm")
                            for h in range(HB):
                                b = HB * B + h
                                for j in range(int(offs[b]) - c0,
                                               int(offs[b + 1]) - c0):
                                    nc.tensor.matmul(
                                        out=pm[h * BW:(h + 1) * BW, :],
                                        lhsT=st[:, j * BW:(j + 1) * BW],
                                        rhs=gt[:, j * D:(j + 1) * D],
                                        start=(j == int(offs[b]) - c0),
                                        stop=(j == int(offs[b + 1]) - c0 - 1),
                                        skip_group_check=True,
                                        tile_position=(0, h * BW))
                            post(sp, B, pm)

            # ----------------- MM layer 1 -----------------------------------
            def mm_l1_post(sp, b, pm):
                tmp = sp.tile([P, D], F32, tag="mtmp")
                nc.vector.tensor_scalar(
                    out=tmp[:], in0=pm[:], scalar1=dinv_t[:, b:b + 1],
                    scalar2=None, op0=mybir.AluOpType.mult)
                m1t = sp.tile([P, D], GATH_DT, tag="m1t")
                nc.scalar.activation(
                    out=m1t[:], in_=tmp[:],
                    func=mybir.ActivationFunctionType.Copy, scale=1.0)
                nc.sync.dma_start(out=m1bf_in[b * P:(b + 1) * P, :], in_=m1t[:])
                nc.vector.tensor_tensor(
                    out=acc_m[:, b * D:(b + 1) * D],
                    in0=acc_m[:, b * D:(b + 1) * D], in1=tmp[:],
                    op=mybir.AluOpType.add)

            spmm_layer(m0f, mm_groups, mm_offs, mmidx_t, mmlr_t, mmval_t, mm_l1_post)
            nc.gpsimd.collective_compute(
                "AllGather", mybir.AluOpType.bypass, replica_groups=RG,
                ins=[m1bf_in[:]], outs=[m1bf_full[:]])

            # ----------------- UI layer 1 -----------------------------------
            def ui_l1_post(sp, b, pm):
                h1t = sp.tile([P, D], GATH_DT, tag="h1t")
                nc.scalar.activation(
                    out=h1t[:], in_=pm[:],
                    func=mybir.ActivationFunctionType.Copy, scale=1.0)
                nc.sync.dma_start(out=h1bf_in[b * P:(b + 1) * P, :], in_=h1t[:])
                nc.vector.tensor_tensor(
                    out=acc_u[:, b * D:(b + 1) * D],
                    in0=acc_u[:, b * D:(b + 1) * D], in1=pm[:],
                    op=mybir.AluOpType.add)

            spmm_layer(x0f, ui_groups, ui_offs, uiidx_t, uilr_t, uival_t, ui_l1_post)
            nc.gpsimd.collective_compute(
                "AllGather", mybir.AluOpType.bypass, replica_groups=RG,
                ins=[h1bf_in[:]], outs=[h1bf_full[:]])

            # ----------------- MM layer 2 -----------------------------------
            def mm_l2_post(sp, b, pm):
                mct = sp.tile([P, D], F32, tag="mct")
                nc.vector.tensor_scalar(
                    out=mct[:], in0=pm[:], scalar1=dinv_t[:, b:b + 1],
                    scalar2=None, op0=mybir.AluOpType.mult)
                nc.vector.tensor_tensor(
                    out=mct[:], in0=acc_m[:, b * D:(b + 1) * D], in1=mct[:],
                    op=mybir.AluOpType.add)
                mbf = sp.tile([P, D], EDGE_DT, tag="mbf")
                nc.scalar.activation(
                    out=mbf[:], in_=mct[:],
                    func=mybir.ActivationFunctionType.Copy,
                    scale=1.0 / (LAYERS + 1))
                nc.sync.dma_start(out=macc_in[b * P:(b + 1) * P, :], in_=mbf[:])

            spmm_layer(m1bf_full, mm_groups, mm_offs, mmidx_t, mmlr_t, mmval_t, mm_l2_post)

            # ----------------- pack + A2A (mashup) ---------------------------
            with tc.tile_pool(name="pkm", bufs=1) as pk:
                pkm_t = pk.tile([P, WM], I32)
                nc.sync.dma_start(out=pkm_t[:], in_=pack_m[:])
                pm_ = pk.tile([P, WM * D], EDGE_DT)
                nc.gpsimd.indirect_dma_start(
                    out=pm_[:], out_offset=None, in_=macc_in[:],
                    in_offset=bass.IndirectOffsetOnAxis(ap=pkm_t[:], axis=0))
                nc.sync.dma_start(
                    out=sm_send[:].rearrange("(w p) d -> p w d", p=P),
                    in_=pm_[:].rearrange("p (w d) -> p w d", d=D))
            nc.gpsimd.collective_compute(
                "AllToAll", mybir.AluOpType.bypass, replica_groups=RG,
                ins=[sm_send[:]], outs=[rm_recv[:]])

            # ----------------- UI layer 2 -----------------------------------
            def ui_l2_post(sp, b, pm):
                uft = sp.tile([P, D], F32, tag="uft")
                nc.vector.tensor_tensor(
                    out=uft[:], in0=acc_u[:, b * D:(b + 1) * D], in1=pm[:],
                    op=mybir.AluOpType.add)
                ubf = sp.tile([P, D], EDGE_DT, tag="ubf")
                nc.scalar.activation(
                    out=ubf[:], in_=uft[:],
                    func=mybir.ActivationFunctionType.Copy,
                    scale=1.0 / (LAYERS + 1))
                nc.sync.dma_start(out=uif_in[b * P:(b + 1) * P, :], in_=ubf[:])

            spmm_layer(h1bf_full, ui_groups, ui_offs, uiidx_t, uilr_t, uival_t, ui_l2_post)

            # ----------------- pack + A2A (uif) ------------------------------
            with tc.tile_pool(name="pku", bufs=1) as pk:
                pku_t = pk.tile([P, WU], I32)
                nc.sync.dma_start(out=pku_t[:], in_=pack_u[:])
                pu = pk.tile([P, WU * D], EDGE_DT)
                nc.gpsimd.indirect_dma_start(
                    out=pu[:], out_offset=None, in_=uif_in[:],
                    in_offset=bass.IndirectOffsetOnAxis(ap=pku_t[:], axis=0))
                nc.sync.dma_start(
                    out=su_send[:].rearrange("(w p) d -> p w d", p=P),
                    in_=pu[:].rearrange("p (w d) -> p w d", d=D))
            nc.gpsimd.collective_compute(
                "AllToAll", mybir.AluOpType.bypass, replica_groups=RG,
                ins=[su_send[:]], outs=[ru_recv[:]])

            # ----------------- head -----------------------------------------
            with tc.tile_pool(name="hd", bufs=1) as hd, \
                 tc.tile_pool(name="hd2", bufs=1) as hd2, \
                 tc.tile_pool(name="hd_ps", bufs=2, space="PSUM") as hps, \
                 tc.tile_pool(name="hd_ps2", bufs=2, space="PSUM") as hps2:
                ident = hd.tile([P, P], BF16)
                make_identity(nc, ident[:])
                ones_t = hd.tile([1, P], F32)
                nc.vector.memset(ones_t[:], 1.0)
                neg_t = hd.tile([1, P], F32)
                nc.vector.memset(neg_t[:], -1e9)
                w1m_t = hd.tile([P, 16], F32)
                nc.sync.dma_start(out=w1m_t[:], in_=aw1[0:D, :])
                w1s_t = hd.tile([P, 16], F32)
                nc.sync.dma_start(out=w1s_t[:], in_=aw1[D:2 * D, :])
                w2_t = hd.tile([16, 1], F32)
                nc.sync.dma_start(out=w2_t[:], in_=aw2[:])
                b1_t = hd.tile([16, 1], F32)
                nc.sync.dma_start(out=b1_t[:], in_=ab1.unsqueeze(1))
                b2_t = hd.tile([1, 1], F32)
                nc.sync.dma_start(out=b2_t[:], in_=ab2.unsqueeze(1))
                b2r_t = hd.tile([P, 1], F32)
                nc.gpsimd.partition_broadcast(b2r_t[:], b2_t[:])
                pw1_t = hd.tile([P, 3 * 8], F32)
                nc.sync.dma_start(
                    out=pw1_t[:].rearrange("p (c h) -> p c h", h=8),
                    in_=pw1.rearrange("(c p) h -> p c h", p=P))
                pb1_t = hd.tile([8, 1], F32)
                nc.sync.dma_start(out=pb1_t[:], in_=pb1.unsqueeze(1))
                pw2_t = hd.tile([8, 1], F32)
                nc.sync.dma_start(out=pw2_t[:], in_=pw2[:])
                pb2_t = hd.tile([1, 1], F32)
                nc.sync.dma_start(out=pb2_t[:], in_=pb2.unsqueeze(1))

                NLB = L * P
                NCK = NLB // 512

                for t in range(BT):
                    midx_t = hd2.tile([P, L], I32, tag="midx")
                    nc.sync.dma_start(out=midx_t[:],
                                      in_=memb_pos[t * P:(t + 1) * P, :])
                    me_t = hd2.tile([P, L * D], EDGE_DT, tag="me")
                    nc.gpsimd.indirect_dma_start(
                        out=me_t[:], out_offset=None, in_=ru_recv[:],
                        in_offset=bass.IndirectOffsetOnAxis(ap=midx_t[:], axis=0))
                    et_t = hd2.tile([P, L * P], F32, tag="et")
                    for l in range(L):
                        ptr = hps.tile([P, P], EDGE_DT, tag="ptr")
                        nc.tensor.transpose(out=ptr[:],
                                            in_=me_t[:, l * D:(l + 1) * D],
                                            identity=ident[:])
                        nc.vector.tensor_copy(out=et_t[:, l * P:(l + 1) * P],
                                              in_=ptr[:])
                    sidx_t = hd2.tile([P, 1], I32, tag="sidx")
                    nc.sync.dma_start(out=sidx_t[:],
                                      in_=svc_pos[t * P:(t + 1) * P, :])
                    sv_t = hd2.tile([P, D], EDGE_DT, tag="sv")
                    nc.gpsimd.indirect_dma_start(
                        out=sv_t[:], out_offset=None, in_=ru_recv[:],
                        in_offset=bass.IndirectOffsetOnAxis(ap=sidx_t[:], axis=0))
                    ptr = hps.tile([P, P], EDGE_DT, tag="ptr")
                    nc.tensor.transpose(out=ptr[:], in_=sv_t[:], identity=ident[:])
                    svcT_t = hd2.tile([P, P], F32, tag="svcT")
                    nc.vector.tensor_copy(out=svcT_t[:], in_=ptr[:])
                    xidx_t = hd2.tile([P, 1], I32, tag="xidx")
                    nc.sync.dma_start(out=xidx_t[:],
                                      in_=mash_pos[t * P:(t + 1) * P, :])
                    ma_t = hd2.tile([P, D], EDGE_DT, tag="ma")
                    nc.gpsimd.indirect_dma_start(
                        out=ma_t[:], out_offset=None, in_=rm_recv[:],
                        in_offset=bass.IndirectOffsetOnAxis(ap=xidx_t[:], axis=0))
                    ptr = hps.tile([P, P], EDGE_DT, tag="ptr")
                    nc.tensor.transpose(out=ptr[:], in_=ma_t[:], identity=ident[:])
                    maT_t = hd2.tile([P, P], F32, tag="maT")
                    nc.vector.tensor_copy(out=maT_t[:], in_=ptr[:])

                    psv = hps2.tile([16, P], F32, tag="ps_small")
                    nc.tensor.matmul(out=psv[:], lhsT=w1s_t[:], rhs=svcT_t[:],
                                     start=True, stop=True)
                    svterm_t = hd2.tile([16, P], F32, tag="svterm")
                    nc.vector.tensor_copy(out=svterm_t[:], in_=psv[:])

                    hdn_t = hd2.tile([16, NLB], F32, tag="hdn")
                    lpc = 512 // P
                    for n in range(NCK):
                        pmt = hps2.tile([16, 512], F32, tag="ps_small")
                        nc.tensor.matmul(out=pmt[:], lhsT=w1m_t[:],
                                         rhs=et_t[:, n * 512:(n + 1) * 512],
                                         start=True, stop=True)
                        tt = hd2.tile([16, 512], F32, tag="tt16")
                        nc.vector.tensor_tensor(
                            out=tt[:].rearrange("h (l b) -> h l b", b=P),
                            in0=pmt[:].rearrange("h (l b) -> h l b", b=P),
                            in1=svterm_t[:].unsqueeze(1).to_broadcast([16, lpc, P]),
                            op=mybir.AluOpType.add)
                        nc.scalar.activation(
                            out=hdn_t[:, n * 512:(n + 1) * 512], in_=tt[:],
                            func=mybir.ActivationFunctionType.Relu,
                            bias=b1_t[:], scale=1.0)

                    sc_t = hd2.tile([1, NLB], F32, tag="sc")
                    for n in range(NCK):
                        pst = hps2.tile([1, 512], F32, tag="ps_small")
                        nc.tensor.matmul(out=pst[:], lhsT=w2_t[:],
                                         rhs=hdn_t[:, n * 512:(n + 1) * 512],
                                         start=True, stop=True)
                        nc.vector.tensor_copy(out=sc_t[:, n * 512:(n + 1) * 512],
                                              in_=pst[:])
                    mk_t = hd2.tile([1, NLB], F32, tag="mk")
                    nc.sync.dma_start(out=mk_t[:], in_=mask_lb[t:t + 1, :])

                    ew_t = hd2.tile([P, NLB], F32, tag="ew")
                    for n in range(NCK):
                        prt = hps.tile([P, 512], F32, tag="prt")
                        nc.tensor.matmul(out=prt[:], lhsT=ones_t[:],
                                         rhs=sc_t[:, n * 512:(n + 1) * 512],
                                         start=True, stop=False)
                        nc.tensor.matmul(out=prt[:], lhsT=neg_t[:],
                                         rhs=mk_t[:, n * 512:(n + 1) * 512],
                                         start=False, stop=True)
                        nc.scalar.activation(
                            out=ew_t[:, n * 512:(n + 1) * 512], in_=prt[:],
                            func=mybir.ActivationFunctionType.Exp,
                            bias=b2r_t[:], scale=1.0)

                    den_t = hd2.tile([P, P], F32, tag="den")
                    nc.vector.tensor_reduce(
                        out=den_t[:],
                        in_=ew_t[:].rearrange("p (l b) -> p b l", b=P),
                        axis=mybir.AxisListType.X, op=mybir.AluOpType.add)
                    rden_t = hd2.tile([P, P], F32, tag="rden")
                    nc.vector.reciprocal(rden_t[:], den_t[:])
                    nc.vector.tensor_tensor(out=ew_t[:], in0=et_t[:], in1=ew_t[:],
                                            op=mybir.AluOpType.mult)
                    gatt_t = hd2.tile([P, P], F32, tag="gatt")
                    nc.vector.tensor_reduce(
                        out=gatt_t[:],
                        in_=ew_t[:].rearrange("p (l b) -> p b l", b=P),
                        axis=mybir.AxisListType.X, op=mybir.AluOpType.add)
                    nc.vector.tensor_tensor(out=gatt_t[:], in0=gatt_t[:],
                                            in1=rden_t[:],
                                            op=mybir.AluOpType.mult)
                    nc.vector.tensor_tensor(out=maT_t[:], in0=gatt_t[:],
                                            in1=maT_t[:], op=mybir.AluOpType.add)
                    elem_t = hd2.tile([P, P], F32, tag="elem")
                    nc.vector.tensor_tensor(out=elem_t[:], in0=maT_t[:],
                                            in1=svcT_t[:],
                                            op=mybir.AluOpType.mult)
                    ppd = hps2.tile([8, P], F32, tag="ps_small")
                    for c, rhs in enumerate((elem_t, maT_t, svcT_t)):
                        nc.tensor.matmul(out=ppd[:], lhsT=pw1_t[:, c * 8:(c + 1) * 8],
                                         rhs=rhs[:], start=(c == 0), stop=(c == 2))
                    hp_t = hd2.tile([8, P], F32, tag="hp")
                    nc.scalar.activation(out=hp_t[:], in_=ppd[:],
                                         func=mybir.ActivationFunctionType.Relu,
                                         bias=pb1_t[:], scale=1.0)
                    pyt = hps2.tile([1, P], F32, tag="ps_small")
                    nc.tensor.matmul(out=pyt[:], lhsT=pw2_t[:], rhs=hp_t[:],
                                     start=True, stop=True)
                    y_t = hd2.tile([1, P], F32, tag="yt")
                    nc.scalar.activation(out=y_t[:], in_=pyt[:],
                                         func=mybir.ActivationFunctionType.Sigmoid,
                                         bias=pb2_t[:], scale=1.0)
                    nc.sync.dma_start(out=y[t * P:(t + 1) * P, :], in_=y_t[:])

    nc.compile()
    return nc


# ---------------------------------------------------------------------------
# host orchestration
# ---------------------------------------------------------------------------

def prepare(inputs, cw=64):
    NU, D = inputs['user_tbl'].shape
    NS = inputs['service_tbl'].shape[0]
    NM = inputs['mashup_tbl'].shape[0]
    B, L = inputs['member_masked'].shape

    NBU = _cdiv(NU + NS, N_CORES * P)
    NBM = _cdiv(NM, N_CORES * P)
    RWU, RWM = NBU * P, NBM * P
    BT = B // (N_CORES * P)
    BC = BT * P

    ui_idx, ui_lr, ui_val, ui_chunks, _ = pack_edges64(
        np.asarray(inputs['adj_rows']), np.asarray(inputs['adj_cols']),
        np.asarray(inputs['adj_vals'], np.float32), RWU // BW,
        NP_BF16, NP_FP8)
    mm_idx, mm_lr, mm_val, mm_chunks, _ = pack_edges64(
        np.asarray(inputs['A_rows']), np.asarray(inputs['A_cols']),
        np.asarray(inputs['A_vals'], np.float32), RWM // BW,
        NP_BF16, NP_FP8)

    x0 = np.zeros((N_CORES * RWU, D), np.float32)
    x0[:NU] = inputs['user_tbl']
    x0[NU:NU + NS] = inputs['service_tbl']
    m0 = np.zeros((N_CORES * RWM, D), np.float32)
    m0[:NM] = inputs['mashup_tbl']
    x0bf = x0.astype(NP_FP8)
    m0bf = m0.astype(NP_FP8)
    dv = np.zeros(N_CORES * RWM, np.float32)
    dv[:NM] = inputs['d_inv']
    dv = dv.reshape(N_CORES, NBM, P).transpose(0, 2, 1).copy()

    mask = np.asarray(inputs['mask'], np.float32).reshape(N_CORES, BT, P, L)
    mask_lb = mask.transpose(0, 1, 3, 2).reshape(N_CORES, BT, L * P).copy()

    # head A2A routing
    memb = np.asarray(inputs['member_masked'], np.int64).reshape(N_CORES, BC, L)
    svc = np.asarray(inputs['service_inputs'], np.int64).reshape(N_CORES, BC)
    mash = np.asarray(inputs['mashup_inputs'], np.int64).reshape(N_CORES, BC)
    # uif-row requests: member rows (user ids) + service rows (NU + svc id)
    req_u = np.concatenate([memb.reshape(N_CORES, -1), NU + svc], axis=1)
    PKU, pack_u, pos_u = build_routing(req_u, RWU)
    memb_pos = pos_u[:, :BC * L].reshape(N_CORES, BC, L).astype(np.int32)
    svc_pos = pos_u[:, BC * L:].reshape(N_CORES, BC, 1).astype(np.int32)
    PKM, pack_m, pos_m = build_routing(mash, RWM)
    mash_pos = pos_m.reshape(N_CORES, BC, 1).astype(np.int32)

    # pack_idx [owner, dest, pkc] -> per-owner tile [P, W] with slot
    # (p, j*(pkc/128)+i) = pack_idx[o, j, i*128+p]
    def pack_tile(pack_idx, pkc):
        W = N_CORES * pkc // P
        t = pack_idx.reshape(N_CORES, N_CORES, pkc // P, P)
        return t.transpose(0, 3, 1, 2).reshape(N_CORES, P, W).copy()

    pu_t = pack_tile(pack_u, PKU)
    pm_t = pack_tile(pack_m, PKM)

    cfg = dict(NU=NU, NS=NS, NM=NM, D=D, L=L, NBU=NBU, NBM=NBM,
               BT=BT, CW=cw, PKU=PKU, PKM=PKM,
               ui_chunks=tuple(int(c) for c in ui_chunks),
               mm_chunks=tuple(int(c) for c in mm_chunks))

    in_maps = []
    for k in range(N_CORES):
        in_maps.append({
            'x0f': x0bf,
            'm0f': m0bf,
            'x0s': x0[k * RWU:(k + 1) * RWU],
            'm0s': m0[k * RWM:(k + 1) * RWM],
            'ui_idx': ui_idx[k], 'ui_lr': ui_lr[k], 'ui_val': ui_val[k],
            'mm_idx': mm_idx[k], 'mm_lr': mm_lr[k], 'mm_val': mm_val[k],
            'dinv': dv[k],
            'pack_u': pu_t[k], 'pack_m': pm_t[k],
            'memb_pos': memb_pos[k],
            'svc_pos': svc_pos[k],
            'mash_pos': mash_pos[k],
            'mask_lb': mask_lb[k],
            'aw1': np.asarray(inputs['att_w1'], np.float32),
            'ab1': np.asarray(inputs['att_b1'], np.float32),
            'aw2': np.asarray(inputs['att_w2'], np.float32),
            'ab2': np.asarray(inputs['att_b2'], np.float32),
            'pw1': np.asarray(inputs['pred_w1'], np.float32),
            'pb1': np.asarray(inputs['pred_b1'], np.float32),
            'pw2': np.asarray(inputs['pred_w2'], np.float32),
            'pb2': np.asarray(inputs['pred_b2'], np.float32),
        })
    return cfg, in_maps


_CACHE = {}


def run(inputs, cw=64, trace=False):
    from concourse.bass_utils import run_bass_kernel_spmd
    cfg, in_maps = prepare(inputs, cw=cw)
    key = tuple(sorted((k, v) for k, v in cfg.items()))
    if key not in _CACHE:
        _CACHE[key] = build_program(cfg)
    nc = _CACHE[key]
    res = run_bass_kernel_spmd(nc, in_maps, core_ids=list(range(N_CORES)),
                               trace=trace)
    yy = np.concatenate([r['y'] for r in res.results], axis=0)
    return yy, res


def kernel(**inputs) -> np.ndarray:
    yy, _ = run(inputs)
    return yy.astype(np.float32)

